# revision 31
# baseline (speedup 1.0000x reference)
"""Multi-head attention with additive positional bias on 8 Trainium2 cores.

Problem: q,k,v [8, 1024, 512] fp32, pos_bias [1, 8, 1024, 1024] fp32,
8 heads x head_dim 64, out = softmax(q@k^T * scale + bias) @ v.

Sharding: one head per NeuronCore (tensor parallel over heads). The bias
table is per-head, so each core only needs its own bias slice.

Per-core layout: compute S^T (scores transposed, j on partitions) so that
  - matmul 1:  S^T[j,i] = sum_d KT[d,j] * QT[d,i]  in bf16, K=128
               zero-padded (K=64 matmuls / PE row tiling wedge this
               runtime).
  - softmax:   exp(S^T) * exp(biasT): 7 j-tiles per batch use the ScalarE
               spline exp then a DVE multiply with resident exp(biasT);
               one tile per batch instead computes exp(s + bias) entirely
               on the Vector engine via a fused Schraudolph bit-trick
                 bf16_bits(exp(s+b)) ~= int16(s*A16 + (b*A16 + B16))
               -- one scalar_tensor_tensor op reading scores from PSUM and
               a resident int16 pre-scaled bias tile; the int16 result is
               bitcast to bf16 and fed straight to matmul 2. This
               rebalances exp work off the otherwise-critical ScalarE and
               costs no DVE bias-multiply for that tile. The Schraudolph
               tile is t=7 (batch tail) except the last batch (t=0) so
               the final drain goes through the shorter ACT path.
               Max-subtraction is skipped (scores ~N(0,1), bias in [-2,2]).
  - matmul 2:  lhsT=[V|ones] tile [j,65], rhs=P^T -> O^T[dv,i] accumulated
               over j tiles in PSUM; the ones-column yields the softmax
               denominators for free in row 64.
Emission is software-pipelined (mm1 two iterations ahead; next batch's
DMAs at batch start) so the PE FIFO never head-of-line blocks on the DVE
multiply. PSUM: score tiles triple-buffered (6 banks) + single O^T
accumulator (2 banks). Startup DMAs are ordered/split so the first score
matmul only waits on ~192KB of wire. All transposes and the final
divide/untranspose are done on the host in numpy.
"""

import numpy as np
from contextlib import ExitStack

import concourse.bacc as bacc
import concourse.bass as bass
import concourse.mybir as mybir
import concourse.tile as tile
from concourse.bass_utils import run_bass_kernel_spmd

B = 8          # batch
S = 1024       # sequence length
D = 512        # model dim
H = 8          # heads
HD = 64        # head dim
NT = S // 128  # 128-row j-tiles per sequence
SCALE = HD ** -0.5

# Schraudolph bf16 exp: bits = int16(s*A16 + b*A16 + B16), bitcast to bf16.
A16 = 128.0 * 1.4426950408889634       # 2^7 * log2(e)
B16 = 16256.0 - 7.0                    # 127*2^7 - c, c tuned for ~zero mean err
# DVE-exp tiles: {6,7} per batch — two tiles' exp moves off ScalarE onto the
# Vector engine, rebalancing the engines (ScalarE was pacer at ~66us busy),
# placed at batch end where the ACT stream has natural slack. The last batch
# uses {0,6} so its tail drains through the short ACT path. (Tried t=0 for
# every batch: the STT queues behind the previous batch's DVE tail — loss.)
SCHRAUD_MAIN = (6, 7)
SCHRAUD_LAST = (0, 6)
BA_SLOTS = {6: 0, 7: 1, 0: 2}          # ba dram slot per Schraudolph tile
ROW_TILED = True                       # K=64 2-way PE row tiling: score chunks stream concurrently

_PROGRAM = None


def _emit(ctx, tc, out, qt, kt, vp, eb, ba, repeat=1):
    nc = tc.nc
    f32 = mybir.dt.float32
    bf16 = mybir.dt.bfloat16
    i16 = mybir.dt.int16

    singles = ctx.enter_context(tc.tile_pool(name="singles", bufs=1))
    qk_pool = ctx.enter_context(tc.tile_pool(name="qk_pool", bufs=2))
    v_pool = ctx.enter_context(tc.tile_pool(name="v_pool", bufs=2))
    # 4 bufs: ACT(x) waits TT(x-4) buf-free instead of TT(x-3) -- the DVE
    # multiply stream lags ~2 slots behind ACT and the batch-edge Schraudolph
    # op pushes it further; 3 bufs exposed that lag as ACT stalls.
    e_pool = ctx.enter_context(tc.tile_pool(name="e_pool", bufs=4))
    p_pool = ctx.enter_context(tc.tile_pool(name="p_pool", bufs=4))
    z_pool = ctx.enter_context(tc.tile_pool(name="z_pool", bufs=3))
    ps_s = ctx.enter_context(tc.tile_pool(name="ps_s", bufs=3, space="PSUM"))
    ps_o = ctx.enter_context(tc.tile_pool(name="ps_o", bufs=1, space="PSUM"))

    eb_tiles = [None] * NT
    nrep = B * repeat
    items = [(r % B, t) for r in range(nrep) for t in range(NT)]

    qk_tiles = {}
    v_tiles = {}

    def schraud_set(r):
        return SCHRAUD_MAIN if r < nrep - 1 else SCHRAUD_LAST

    def issue_dmas(r):
        b = r % B
        qtb = qk_pool.tile([128, S], bf16, tag="qtb", name=f"qtb{r}")
        ktb = qk_pool.tile([128, S], bf16, tag="ktb", name=f"ktb{r}")
        vpb = v_pool.tile([128, NT, HD + 1], bf16, tag="vpb", name=f"vpb{r}")
        if r == 0:
            # Fine-grained + prioritized so mm1(t=0) waits on ~192KB, not 2MB:
            # KT cols for tiles 0-1, then QT halves, then the rest.
            nc.sync.dma_start(out=ktb[:, :256], in_=kt[b][:, :256])
            nc.sync.dma_start(out=qtb[:, :512], in_=qt[b][:, :512])
            nc.sync.dma_start(out=qtb[:, 512:], in_=qt[b][:, 512:])
            nc.sync.dma_start(out=ktb[:, 256:], in_=kt[b][:, 256:])
        else:
            nc.sync.dma_start(out=qtb, in_=qt[b])
            nc.sync.dma_start(out=ktb, in_=kt[b])
        nc.sync.dma_start(out=vpb, in_=vp[b])
        qk_tiles[r] = (qtb, ktb)
        v_tiles[r] = vpb

    def emit_mm1(idx):
        r = idx // NT
        t = idx % NT
        qtb, ktb = qk_tiles[r]
        ps = ps_s.tile([128, S], f32, tag="ps", name=f"ps{idx}")
        for c in range(2):
            cs = slice(c * 512, (c + 1) * 512)
            # S^T chunk: [j=128, i=512] = KT_tile.T @ QT_chunk. Row-tiled:
            # chunk c uses K=64 on PE row group 64*c (QT/KT duplicated on
            # both partition halves), so the two chunks stream concurrently.
            rs = slice(64 * c, 64 * c + HD) if ROW_TILED else slice(0, 128)
            nc.tensor.matmul(
                ps[:, cs],
                ktb[rs, t * 128:(t + 1) * 128],
                qtb[rs, cs],
                start=True,
                stop=True,
            )
        return ps

    issue_dmas(0)
    # exp(bias^T) resident in SBUF as ONE [128, 8, 1024] tile (16KB/
    # partition bf16) so any two adjacent j-tiles form a contiguous
    # [128, 2048] slice for paired DVE multiplies. Issued after batch 0's
    # q/k in first-use order; pre-scaled int16 bias tiles for the
    # Schraudolph path last (first used at batch 0, t=7).
    eb_all = singles.tile([128, NT, S], bf16, name="eb_all")
    for t in range(NT):
        nc.sync.dma_start(out=eb_all[:, t, :], in_=eb[t * 128:(t + 1) * 128, :])
        eb_tiles[t] = eb_all[:, t, :]
    ba_tiles = {}
    for t, slot in BA_SLOTS.items():
        bat = singles.tile([128, S], i16, name=f"ba{t}")
        nc.sync.dma_start(out=bat, in_=ba[slot])
        ba_tiles[t] = bat

    ps_tiles = {0: emit_mm1(0), 1: emit_mm1(1)}
    po = None
    pend = None  # (first_t, ebf2 pair tile) awaiting its partner ACT tile

    def emit_mm2(r, t, pbf_cs):
        vpb = v_tiles[r]
        for c in range(2):
            cs = slice(c * 512, (c + 1) * 512)
            # O^T accum: [dv=65, i=512] += Vpad_tile.T @ P^T_chunk
            nc.tensor.matmul(
                po[:, cs],
                vpb[:, t, :],
                pbf_cs(c),
                start=(t == 0),
                stop=(t == NT - 1),
            )

    def flush_pair(r, n):
        # One DVE multiply over n adjacent exp tiles ([128, n*1024] dense).
        t0, ebf2 = pend
        pbf2 = p_pool.tile([128, 2, S], bf16, tag="pbf", name=f"pbf{r}_{t0}")
        nc.vector.tensor_mul(
            pbf2[:, :n, :], ebf2[:, :n, :], eb_all[:, t0:t0 + n, :]
        )
        for j in range(n):
            emit_mm2(r, t0 + j, lambda c, j=j: pbf2[:, j, c * 512:(c + 1) * 512])

    def emit_evac(r, b, po_r):
        # PV-accumulator evacuation on ScalarE (freed by the Schraudolph
        # offload); DVE stays dedicated to the exp/mul stream.
        osb = p_pool.tile([HD + 1, S], f32, tag="osb", name=f"osb{r}")
        for c in range(2):
            cs = slice(c * 512, (c + 1) * 512)
            nc.scalar.copy(osb[:, cs], po_r[:, cs])
            nc.sync.dma_start(out=out[b][:, cs], in_=osb[:, cs])

    evac_q = []
    for idx, (b, t) in enumerate(items):
        r = idx // NT
        while evac_q and evac_q[0][0] <= idx:
            _, er, eb_, epo = evac_q.pop(0)
            emit_evac(er, eb_, epo)
        if t == 0:
            if r + 1 < nrep:
                issue_dmas(r + 1)
            po = ps_o.tile([HD + 1, S], f32, tag="po", name=f"po{r}")
        if idx + 2 < len(items):
            ps_tiles[idx + 2] = emit_mm1(idx + 2)
        ps = ps_tiles.pop(idx)

        if t in schraud_set(r):
            if pend is not None:
                # Flush the odd ACT tile before the Schraudolph tile's mm2
                # (which may carry the accumulation stop flag).
                flush_pair(r, 1)
                pend = None
            # exp(s + bias) in one fused DVE op: int16(s*A16 + ba), ba int16
            # pre-scaled on host; bits reinterpreted as bf16 feed matmul 2.
            zi = z_pool.tile([128, S], i16, tag="zi", name=f"zi{idx}")
            nc.vector.scalar_tensor_tensor(
                zi, ps, A16, ba_tiles[t],
                mybir.AluOpType.mult, mybir.AluOpType.add,
            )
            zb = zi.bitcast(bf16)
            emit_mm2(r, t, lambda c: zb[:, c * 512:(c + 1) * 512])
        else:
            if pend is None:
                ebf2 = e_pool.tile([128, 2, S], bf16, tag="ebf", name=f"ebf{idx}")
                pend = (t, ebf2)
                half = 0
            else:
                ebf2 = pend[1]
                half = 1
            nc.scalar.activation(
                ebf2[:, half, :], ps, mybir.ActivationFunctionType.Exp
            )
            if half == 1:
                flush_pair(r, 2)
                pend = None
        if t == NT - 1 and pend is not None:
            flush_pair(r, 1)
            pend = None

        if t == NT - 1:
            # Defer evacuation emission 3 iterations so the ScalarE copies
            # never head-of-line block the next batch's first ACTIVATEs
            # while waiting on this batch's final PV matmul.
            evac_q.append((idx + 3, r, b, po))
    while evac_q:
        _, er, eb_, epo = evac_q.pop(0)
        emit_evac(er, eb_, epo)


def _build_program(repeat=1):
    nc = bacc.Bacc("TRN2", target_bir_lowering=False, debug=False)
    qt = nc.dram_tensor("qt", [B, 128, S], mybir.dt.bfloat16, kind="ExternalInput").ap()
    kt = nc.dram_tensor("kt", [B, 128, S], mybir.dt.bfloat16, kind="ExternalInput").ap()
    vp = nc.dram_tensor(
        "vp", [B, 128, NT, HD + 1], mybir.dt.bfloat16, kind="ExternalInput"
    ).ap()
    eb = nc.dram_tensor("eb", [S, S], mybir.dt.bfloat16, kind="ExternalInput").ap()
    ba = nc.dram_tensor("ba", [3, 128, S], mybir.dt.int16, kind="ExternalInput").ap()
    out = nc.dram_tensor("out", [B, HD + 1, S], mybir.dt.float32, kind="ExternalOutput").ap()
    with tile.TileContext(nc) as tc, ExitStack() as ctx:
        _emit(ctx, tc, out, qt, kt, vp, eb, ba, repeat=repeat)
    nc.compile()
    return nc


def get_program(repeat=1):
    global _PROGRAM
    if repeat != 1:
        return _build_program(repeat)
    if _PROGRAM is None:
        _PROGRAM = _build_program()
    return _PROGRAM


def make_in_maps(q, k, v, pos_bias):
    import ml_dtypes

    bf = ml_dtypes.bfloat16
    q4 = q.reshape(B, S, H, HD)
    k4 = k.reshape(B, S, H, HD)
    v4 = v.reshape(B, S, H, HD)
    ones = np.ones((B, S, 1), np.float32)
    in_maps = []
    for h in range(H):
        # QT/KT on partition halves: duplicated for 2-way PE row tiling,
        # zero-padded for the K=128 fallback.
        qt = np.zeros((B, 128, S), bf)
        qt[:, :HD, :] = (q4[:, :, h, :].transpose(0, 2, 1) * np.float32(SCALE)).astype(bf)
        kt = np.zeros((B, 128, S), bf)
        kt[:, :HD, :] = k4[:, :, h, :].transpose(0, 2, 1).astype(bf)
        if ROW_TILED:
            qt[:, HD:, :] = qt[:, :HD, :]
            kt[:, HD:, :] = kt[:, :HD, :]
        vp = np.concatenate([v4[:, :, h, :], ones], axis=2)  # [B, S, 65]
        vp = np.ascontiguousarray(
            vp.reshape(B, NT, 128, HD + 1).transpose(0, 2, 1, 3)
        ).astype(bf)  # [B, 128, NT, 65]
        bT = pos_bias[0, h].T.astype(np.float32)  # [S(j), S(i)]
        eb = np.exp(bT).astype(bf)
        ba = np.empty((3, 128, S), np.int16)
        for tt, slot in BA_SLOTS.items():
            ba[slot] = np.rint(
                bT[tt * 128:(tt + 1) * 128, :] * np.float32(A16) + np.float32(B16)
            ).astype(np.int16)
        in_maps.append({"qt": qt, "kt": kt, "vp": vp, "eb": eb, "ba": ba})
    return in_maps


def assemble_output(results):
    out = np.empty((B, S, D), np.float32)
    for h in range(H):
        o = results[h]["out"]  # [B, 65, S]
        normed = o[:, :HD, :] / o[:, HD:HD + 1, :]
        out[:, :, h * HD:(h + 1) * HD] = normed.transpose(0, 2, 1)
    return out


def kernel(q, k, v, pos_bias):
    nc = get_program()
    in_maps = make_in_maps(
        np.asarray(q, np.float32),
        np.asarray(k, np.float32),
        np.asarray(v, np.float32),
        np.asarray(pos_bias, np.float32),
    )
    res = run_bass_kernel_spmd(nc, in_maps, list(range(H))).results
    return assemble_output(res)
